# revision 21
# baseline (speedup 1.0000x reference)
"""Trainium2 Bass kernel: 2-layer Chebyshev graph conv (K=5) + 3-layer MLP head.

Distribution over 8 NeuronCores (row-sharded adjacency + AllGather):
  - Core i owns graph rows [1024*i, 1024*(i+1)).  Its a^T shard is cast to
    fp16 and kept RESIDENT in SBUF (16.8 MB) for all 8 graph applications,
    loaded once per pass.
  - All graph matmuls run adjacency-stationary: lhsT = a^T chunk
    [128, 128] (fp16, FWL), rhs = T chunk streamed from gathered
    node-major state.  The contraction order over nodes is PERMUTED so
    that every gather-readback DMA is per-partition contiguous:
    contraction partition p < 64 covers local rows [0, 512) of every
    rank, p >= 64 covers rows [512, 1024) -- so the per-HALF gather
    buffers of the split AllGather are each read by one contiguous
    partition range.
  - conv2's output rows are computed in two j-halves, each swept over
    all 64 contraction chunks; the first half's AllGather runs while
    the second half's matmuls stream, hiding collective latency (the
    dominant cost at ~200 us per fp16 AllGather in this runtime).
  - Optionally (KER_CC8=1, default) the large AllGather payloads
    (h and conv2 T_k) are cast fp16->fp8e4m3 by the vector engine
    before the collective and cast back after, halving collective
    bytes.  Chebyshev recursion state stays fp16/f32 locally.
  - fc1 contracts over the local node shard (weights host-permuted so
    every fc1 weight DMA is a contiguous 512 KB read), partial
    [16, 512] results are AllReduced, then fc2/fc3/softmax run
    redundantly on every core.
"""

import os
import sys

import numpy as np

for _p in ("/opt/trn_rl_repo", "/root/.axon_site/_ro/trn_rl_repo"):
    if os.path.isdir(_p) and _p not in sys.path:
        sys.path.insert(0, _p)

P = 128            # SBUF partitions
N = 8192           # nodes
B = 16             # batch
F_IN = 2
F1 = 32
F2 = 32
K = 5              # Chebyshev order
NCORES = 8
R = N // NCORES    # nodes per core (1024)
Q = N // P         # contraction chunks (64)
NJ = R // P        # local node blocks (8)
HJ = NJ // 2       # j-blocks per half (4)
HR = R // 2        # rows per half (512)
C1 = B * F_IN      # conv1 state width (32)
C2 = B * F1        # conv2 state width (512)
M1, M2, M3 = 512, 128, 2
QB = 4             # conv2 moving-operand chunk batch (4 KB DMA lines)
FB = 4             # fc1 f2-batch (512 KB DMA reads)

CC8 = os.environ.get("KER_CC8", "0") == "1"   # fp8 payloads for big AllGathers
                                              # (fails the 2e-2 gate: rel_err 2.8e-2)
REPEAT = int(os.environ.get("KER_REPEAT", "1"))

_CACHE = {}


def build_kernel(repeat=None, cc8=None):
    from concourse import bacc, mybir, tile
    from concourse.masks import make_identity

    REP = repeat if repeat is not None else REPEAT
    CC8_ = CC8 if cc8 is None else cc8

    dt = mybir.dt
    f32 = dt.float32
    f32r = dt.float32r
    f16 = dt.float16
    f8 = dt.float8e4
    cc_dt = f8 if CC8_ else f16
    Alu = mybir.AluOpType
    Act = mybir.ActivationFunctionType
    RG = [list(range(NCORES))]

    nc = bacc.Bacc(
        "TRN2",
        target_bir_lowering=False,
        debug=False,
        enable_asserts=False,
        num_devices=NCORES,
    )

    # ------------------------- DRAM I/O -------------------------
    at4_d = nc.dram_tensor("at4", [N, R], f16, kind="ExternalInput").ap()
    x2dh_d = nc.dram_tensor("x2dh", [N, C1], f16, kind="ExternalInput").ap()
    xloc_d = nc.dram_tensor("xloc", [P, NJ * C1], f16, kind="ExternalInput").ap()
    w1bd_d = nc.dram_tensor("w1bd", [C1, K * C2], f16, kind="ExternalInput").ap()
    w2bd_d = nc.dram_tensor("w2bd", [P, K * P], f16, kind="ExternalInput").ap()
    fw1s_d = nc.dram_tensor("fw1s", [NJ * F2 * P, M1], f16, kind="ExternalInput").ap()
    fw2_d = nc.dram_tensor("fw2", [M1, M2], f32r, kind="ExternalInput").ap()
    fw3_d = nc.dram_tensor("fw3", [M2, M3], f32r, kind="ExternalInput").ap()
    b1r_d = nc.dram_tensor("b1r", [P, C2], f32, kind="ExternalInput").ap()
    b2r_d = nc.dram_tensor("b2r", [P, C2], f32, kind="ExternalInput").ap()
    fb1r_d = nc.dram_tensor("fb1r", [B, M1], f32, kind="ExternalInput").ap()
    fb2r_d = nc.dram_tensor("fb2r", [B, M2], f32, kind="ExternalInput").ap()
    fb3r_d = nc.dram_tensor("fb3r", [B, M3], f32, kind="ExternalInput").ap()
    out_d = nc.dram_tensor("out", [B, M3], f32, kind="ExternalOutput").ap()

    with tile.TileContext(nc) as tc:
        with (
            tc.tile_pool(name="consts", bufs=1) as consts,
            tc.tile_pool(name="atp", bufs=1) as atp,
            tc.tile_pool(name="c1p", bufs=2) as c1p,
            tc.tile_pool(name="tfpp", bufs=2) as tfpp,
            tc.tile_pool(name="o1p", bufs=1) as o1p,
            tc.tile_pool(name="tf2p", bufs=2) as tf2p,
            tc.tile_pool(name="t8p", bufs=2) as t8p,
            tc.tile_pool(name="ttp", bufs=2) as ttp,
            tc.tile_pool(name="fwp", bufs=2) as fwp,
            tc.tile_pool(name="fcp", bufs=1) as fcp,
            tc.tile_pool(name="psum", bufs=8, space="PSUM") as psp,
            tc.tile_pool(name="dram", bufs=2, space="DRAM") as drp,
        ):
            # ------------------------- constants -------------------------
            ident32 = consts.tile([32, 32], f32)
            make_identity(nc, ident32)
            ident16 = consts.tile([P, P], f16)
            make_identity(nc, ident16)
            w1bd = consts.tile([C1, K * C2], f16)
            nc.scalar.dma_start(out=w1bd[:], in_=w1bd_d[:])
            w2bd = consts.tile([P, K * P], f16)
            nc.scalar.dma_start(out=w2bd[:], in_=w2bd_d[:])
            b1r = consts.tile([P, C2], f32)
            nc.scalar.dma_start(out=b1r[:], in_=b1r_d[:])
            b2r = consts.tile([P, C2], f32)
            nc.scalar.dma_start(out=b2r[:], in_=b2r_d[:])
            fb1r = consts.tile([B, M1], f32)
            nc.scalar.dma_start(out=fb1r[:], in_=fb1r_d[:])
            fb2r = consts.tile([B, M2], f32)
            nc.scalar.dma_start(out=fb2r[:], in_=fb2r_d[:])
            fb3r = consts.tile([B, M3], f32)
            nc.scalar.dma_start(out=fb3r[:], in_=fb3r_d[:])
            fw3sb = consts.tile([M2, M3], f32r)
            nc.scalar.dma_start(out=fw3sb[:], in_=fw3_d[:])

            def gather_read(dst_tile, src, width):
                """Read gathered [N, width] node-major DRAM (rank blocks of
                1024 rows) into dst [128, Q*width] under the half-based
                contraction permutation: partition p<64 <- rows
                r*1024 + jh*64 + q, p>=64 <- rows r*1024 + 512 + jh*64 + q.
                One DMA per (half, rank): [8 partitions, Q*width]."""
                src_v = src.rearrange(
                    "(r h jh q) f -> h r jh q f", h=2, jh=NJ, q=Q
                )
                for hh in range(2):
                    for r in range(NCORES):
                        p0 = hh * 64 + r * NJ
                        eng = nc.sync if (hh * NCORES + r) % 2 else nc.scalar
                        eng.dma_start(
                            out=dst_tile[p0:p0 + NJ, :].rearrange(
                                "p (q f) -> p q f", f=width
                            ),
                            in_=src_v[hh, r],
                        )

            def emit_body(rep):
                # ---- resident a^T shard: 16 x 1 MB DMAs, consumed chunkwise
                atr = atp.tile([P, Q * R], f16, tag="atr", name=f"atr_{rep}")
                at4_v = at4_d.rearrange("(q p) r -> p q r", p=P)
                atr_v = atr.rearrange("p (q r) -> p q r", r=R)
                for g in range(16):
                    nc.sync.dma_start(
                        out=atr_v[:, g * 4:(g + 1) * 4, :],
                        in_=at4_v[:, g * 4:(g + 1) * 4, :],
                    )

                # ---- conv1 locals
                xloc = c1p.tile([P, NJ * C1], f16, tag="xloc", name=f"xloc_{rep}")
                nc.scalar.dma_start(out=xloc[:], in_=xloc_d[:])
                t_odd = c1p.tile([P, NJ * C1], f16, tag="todd", name=f"todd_{rep}")
                tfP = tfpp.tile([P, Q * C1], f16, tag="tfp", name=f"tfp0_{rep}")
                # x2dh is host-permuted to "(p q)" contraction order: one DMA
                nc.scalar.dma_start(
                    out=tfP.rearrange("p (q f) -> p q f", f=C1),
                    in_=x2dh_d.rearrange("(p q) f -> p q f", q=Q),
                )
                out1 = o1p.tile([P, NJ * C2], f16, tag="out1", name=f"out1_{rep}")

                def c1_feature(state, k, js=range(NJ)):
                    """out1[j] (+)= transpose(state[:, j]) @ w1bd[k]"""
                    for j in js:
                        tt_ps = psp.tile([C1, P], f16, tag="ps", name=f"c1t_{rep}_{k}_{j}")
                        nc.tensor.transpose(
                            tt_ps[:], state[:, j * C1:(j + 1) * C1], ident16[:]
                        )
                        tt = ttp.tile([C1, P], f16, tag="tt", name=f"c1tt_{rep}_{k}_{j}")
                        nc.vector.tensor_copy(tt[:], tt_ps[:])
                        f_ps = psp.tile([P, C2], f32, tag="ps", name=f"c1f_{rep}_{k}_{j}")
                        nc.tensor.matmul(
                            f_ps[:], tt[:], w1bd[:, k * C2:(k + 1) * C2],
                            start=True, stop=True,
                        )
                        sl = out1[:, j * C2:(j + 1) * C2]
                        if k == 0:
                            nc.vector.tensor_copy(sl, f_ps[:])
                        else:
                            nc.vector.tensor_add(sl, sl, f_ps[:])

                c1_feature(xloc, 0)

                for k in range(1, K):
                    ps_g = [
                        psp.tile([P, C1], f32, tag="ps", name=f"c1g_{rep}_{k}_{j}")
                        for j in range(NJ)
                    ]
                    for q in range(Q):
                        rhs = tfP[:, q * C1:(q + 1) * C1]
                        for j in range(NJ):
                            nc.tensor.matmul(
                                ps_g[j][:],
                                atr[:, q * R + j * P: q * R + (j + 1) * P],
                                rhs,
                                start=(q == 0),
                                stop=(q == Q - 1),
                            )
                    dst = t_odd if k % 2 == 1 else xloc
                    for j in range(NJ):
                        sl = dst[:, j * C1:(j + 1) * C1]
                        if k == 1:
                            nc.vector.tensor_copy(sl, ps_g[j][:])
                        else:
                            nc.vector.scalar_tensor_tensor(
                                sl, ps_g[j][:], 2.0, sl, Alu.mult, Alu.subtract
                            )
                    if k < K - 1:
                        cc_in = drp.tile([R, C1], f16, tag="cc1i", name=f"cc1i_{rep}_{k}")
                        nc.scalar.dma_start(
                            out=cc_in.rearrange("(j p) f -> p j f", p=P),
                            in_=dst.rearrange("p (j f) -> p j f", f=C1),
                        )
                        cc_out = drp.tile(
                            [N, C1], f16, tag="cc1o", addr_space="Shared",
                            name=f"cc1o_{rep}_{k}",
                        )
                        nc.gpsimd.collective_compute(
                            "AllGather", Alu.bypass, replica_groups=RG,
                            ins=[cc_in.opt()], outs=[cc_out.opt()],
                        )
                        tfP = tfpp.tile([P, Q * C1], f16, tag="tfp", name=f"tfp_{rep}_{k}")
                        gather_read(tfP, cc_out, C1)
                    c1_feature(dst, k)

                # ---- split spill + AllGather of a [P, NJ*C2] state tile:
                # half hh covers j-blocks [hh*HJ, (hh+1)*HJ) = local rows
                # [hh*512, (hh+1)*512); payload optionally fp8.
                def spill_ag_half(state, hh, tagb, name):
                    sl = state[:, hh * HJ * C2:(hh + 1) * HJ * C2]
                    if CC8_:
                        st8 = t8p.tile(
                            [P, HJ * C2], f8, tag="st8", name=f"st8_{name}_{hh}"
                        )
                        nc.vector.tensor_copy(st8[:], sl)
                        sl = st8[:]
                    cc_in = drp.tile(
                        [HR, C2], cc_dt, tag=f"ci{tagb}{hh}", name=f"ci_{name}_{hh}"
                    )
                    nc.scalar.dma_start(
                        out=cc_in.rearrange("(j p) f -> p j f", p=P),
                        in_=sl.rearrange("p (j f) -> p j f", f=C2),
                    )
                    cc_out = drp.tile(
                        [NCORES * HR, C2], cc_dt, tag=f"co{tagb}{hh}",
                        addr_space="Shared", name=f"co_{name}_{hh}",
                    )
                    nc.gpsimd.collective_compute(
                        "AllGather", Alu.bypass, replica_groups=RG,
                        ins=[cc_in.opt()], outs=[cc_out.opt()],
                    )
                    return cc_out

                # conv1 epilogue per half: h = relu(out1 + b1), spill + AG
                src_h = [None, None]
                for hh in range(2):
                    for j in range(hh * HJ, (hh + 1) * HJ):
                        sl = out1[:, j * C2:(j + 1) * C2]
                        nc.vector.tensor_add(sl, sl, b1r[:])
                        nc.vector.tensor_relu(sl, sl)
                    src_h[hh] = spill_ag_half(out1, hh, "h", f"h_{rep}")

                # ---- conv2
                locB = o1p.tile([P, NJ * C2], f16, tag="locB", name=f"locB_{rep}")
                out2 = o1p.tile([P, NJ * C2], f16, tag="out2", name=f"out2_{rep}")

                def c2_feature(state, k, js):
                    """out2[j] (+)= transpose(state[:, j]) @ w2bd[k] (block-diag)"""
                    for j in js:
                        f_ps = psp.tile([P, C2], f32, tag="ps", name=f"c2f_{rep}_{k}_{j}")
                        for c in range(4):
                            tt_ps = psp.tile(
                                [P, P], f16, tag="ps", name=f"c2t_{rep}_{k}_{j}_{c}"
                            )
                            nc.tensor.transpose(
                                tt_ps[:],
                                state[:, j * C2 + c * P: j * C2 + (c + 1) * P],
                                ident16[:],
                            )
                            tt = ttp.tile([P, P], f16, tag="tt", name=f"c2tt_{rep}_{k}_{j}_{c}")
                            nc.vector.tensor_copy(tt[:], tt_ps[:])
                            nc.tensor.matmul(
                                f_ps[:, c * P:(c + 1) * P],
                                tt[:],
                                w2bd[:, k * P:(k + 1) * P],
                                start=True,
                                stop=True,
                            )
                        sl = out2[:, j * C2:(j + 1) * C2]
                        if k == 0:
                            nc.vector.tensor_copy(sl, f_ps[:])
                        else:
                            nc.vector.tensor_add(sl, sl, f_ps[:])

                c2_feature(out1, 0, range(NJ))

                def load_tft(srcs, nm, qb):
                    """One QB-chunk batch of the gathered state, fp8->fp16
                    cast on DVE if CC8."""
                    raw = (
                        t8p.tile([P, QB * C2], f8, tag="tf8", name=f"tf8_{rep}_{nm}")
                        if CC8_
                        else tf2p.tile([P, QB * C2], f16, tag="tf2", name=f"tf2_{rep}_{nm}")
                    )
                    for hh in range(2):
                        src_v = srcs[hh].rearrange("(p q) f -> p q f", q=Q)
                        nc.sync.dma_start(
                            out=raw[hh * 64:(hh + 1) * 64, :].rearrange(
                                "p (qq f) -> p qq f", f=C2
                            ),
                            in_=src_v[:, qb * QB:(qb + 1) * QB, :],
                        )
                    if CC8_:
                        tft = tf2p.tile(
                            [P, QB * C2], f16, tag="tf2", name=f"tf2c_{rep}_{nm}"
                        )
                        nc.vector.tensor_copy(tft[:], raw[:])
                        return tft
                    return raw

                srcs = src_h
                for k in range(1, K):
                    dst = locB if k % 2 == 1 else out1
                    ps_g = [
                        psp.tile([P, C2], f32, tag="ps", name=f"c2g_{rep}_{k}_{j}")
                        for j in range(NJ)
                    ]
                    for qb in range(Q // QB):
                        tft = load_tft(srcs, f"{k}_{qb}", qb)
                        for qq in range(QB):
                            q = qb * QB + qq
                            rhs = tft[:, qq * C2:(qq + 1) * C2]
                            for j in range(NJ):
                                nc.tensor.matmul(
                                    ps_g[j][:],
                                    atr[:, q * R + j * P: q * R + (j + 1) * P],
                                    rhs,
                                    start=(q == 0),
                                    stop=(q == Q - 1),
                                )
                    for j in range(NJ):
                        sl = dst[:, j * C2:(j + 1) * C2]
                        if k == 1:
                            nc.vector.tensor_copy(sl, ps_g[j][:])
                        else:
                            nc.vector.scalar_tensor_tensor(
                                sl, ps_g[j][:], 2.0, sl, Alu.mult, Alu.subtract
                            )
                    if k < K - 1:
                        srcs = [
                            spill_ag_half(dst, hh, "t", f"t{k}_{rep}")
                            for hh in range(2)
                        ]
                    c2_feature(dst, k, range(NJ))

                # conv2 epilogue: h2 = relu(out2 + b2)  (fp16, fc1 lhs)
                for j in range(NJ):
                    sl = out2[:, j * C2:(j + 1) * C2]
                    nc.vector.tensor_add(sl, sl, b2r[:])
                    nc.vector.tensor_relu(sl, sl)

                # ---- fc1 (node-sharded contraction) + AllReduce
                fw1_v = fw1s_d.rearrange(
                    "(j f c p) m -> j f p c m", p=P, c=FB, f=F2 // FB
                )
                fc_ps = psp.tile([B, M1], f32, tag="ps", name=f"fc1_{rep}")
                n_mm = NJ * F2
                i_mm = 0
                for j in range(NJ):
                    lhs_j = out2[:, j * C2:(j + 1) * C2].rearrange(
                        "p (b f) -> p f b", f=F2
                    )
                    for fb in range(F2 // FB):
                        fwt = fwp.tile(
                            [P, FB * M1], f16, tag="fw", name=f"fw1_{rep}_{j}_{fb}"
                        )
                        nc.sync.dma_start(
                            out=fwt.rearrange("p (c m) -> p c m", m=M1),
                            in_=fw1_v[j, fb],
                        )
                        for ff in range(FB):
                            f = fb * FB + ff
                            nc.tensor.matmul(
                                fc_ps[:],
                                lhs_j[:, f, :],
                                fwt[:, ff * M1:(ff + 1) * M1],
                                start=(i_mm == 0),
                                stop=(i_mm == n_mm - 1),
                            )
                            i_mm += 1

                z = fcp.tile([B, M1], f32, tag="z", name=f"z_{rep}")
                nc.vector.tensor_copy(z[:], fc_ps[:])
                cc_fi = drp.tile([B, M1], f32, tag="ccfi", name=f"ccfi_{rep}")
                nc.scalar.dma_start(out=cc_fi[:], in_=z[:])
                cc_fo = drp.tile(
                    [B, M1], f32, tag="ccfo", addr_space="Shared", name=f"ccfo_{rep}"
                )
                nc.gpsimd.collective_compute(
                    "AllReduce", Alu.add, replica_groups=RG,
                    ins=[cc_fi.opt()], outs=[cc_fo.opt()],
                )

                # z1 = relu(fc1 + fb1), padded to 32 partitions for PE transpose
                z1p = fcp.tile([32, M1], f32, tag="z1p", name=f"z1p_{rep}")
                nc.vector.memset(z1p[:], 0.0)
                nc.scalar.dma_start(out=z1p[0:B, :], in_=cc_fo[:])
                nc.vector.tensor_add(z1p[0:B, :], z1p[0:B, :], fb1r[:])
                nc.vector.tensor_relu(z1p[0:B, :], z1p[0:B, :])

                # fc2
                fc2_ps = psp.tile([B, M2], f32, tag="ps", name=f"fc2_{rep}")
                for c in range(4):
                    zt_ps = psp.tile([P, 32], f32, tag="ps", name=f"zt_{rep}_{c}")
                    nc.tensor.transpose(
                        zt_ps[:], z1p[:, c * P:(c + 1) * P], ident32[:]
                    )
                    zt = fcp.tile([P, 32], f32r, tag="zt", name=f"ztc_{rep}_{c}")
                    nc.vector.tensor_copy(zt[:], zt_ps[:])
                    fwt2 = fcp.tile([P, M2], f32r, tag="fw2t", name=f"fw2t_{rep}_{c}")
                    nc.scalar.dma_start(out=fwt2[:], in_=fw2_d[c * P:(c + 1) * P, :])
                    nc.tensor.matmul(
                        fc2_ps[:], zt[:, 0:B], fwt2[:], start=(c == 0), stop=(c == 3)
                    )
                z2p = fcp.tile([32, M2], f32, tag="z2p", name=f"z2p_{rep}")
                nc.vector.memset(z2p[:], 0.0)
                nc.vector.tensor_copy(z2p[0:B, :], fc2_ps[:])
                nc.vector.tensor_add(z2p[0:B, :], z2p[0:B, :], fb2r[:])
                nc.vector.tensor_relu(z2p[0:B, :], z2p[0:B, :])

                # fc3
                z3t_ps = psp.tile([P, 32], f32, tag="ps", name=f"z3t_{rep}")
                nc.tensor.transpose(z3t_ps[:], z2p[:], ident32[:])
                z3t = fcp.tile([P, 32], f32r, tag="z3t", name=f"z3tc_{rep}")
                nc.vector.tensor_copy(z3t[:], z3t_ps[:])
                fc3_ps = psp.tile([B, M3], f32, tag="ps", name=f"fc3_{rep}")
                nc.tensor.matmul(
                    fc3_ps[:], z3t[:, 0:B], fw3sb[:], start=True, stop=True
                )
                s = fcp.tile([B, M3], f32, tag="s", name=f"s_{rep}")
                nc.vector.tensor_copy(s[:], fc3_ps[:])
                nc.vector.tensor_add(s[:], s[:], fb3r[:])

                # softmax over the last dim (M3 = 2)
                mx = fcp.tile([B, 1], f32, tag="mx", name=f"mx_{rep}")
                nc.vector.reduce_max(mx[:], s[:], axis=mybir.AxisListType.X)
                nc.vector.tensor_scalar_mul(mx[:], mx[:], -1.0)
                nc.scalar.activation(s[:], s[:], Act.Exp, bias=mx[:, 0:1])
                sm = fcp.tile([B, 1], f32, tag="sm", name=f"sm_{rep}")
                nc.vector.reduce_sum(sm[:], s[:], axis=mybir.AxisListType.X)
                nc.vector.reciprocal(sm[:], sm[:])
                nc.vector.tensor_scalar_mul(s[:], s[:], sm[:, 0:1])
                nc.sync.dma_start(out=out_d[:], in_=s[:])

            for _rep in range(REP):
                emit_body(_rep)

    nc.compile()
    return nc


def _m_of():
    p = np.arange(P)
    hm = p >= 64
    pb = np.where(hm, p - 64, p)
    base = (pb >> 3) * R + np.where(hm, HR, 0) + (pb & 7) * Q
    return base[:, None] + np.arange(Q)[None, :]      # [p, q] -> node m


def _contraction_perm():
    """perm[q*128+p] = node index m(p, q) (q-major, for at4 rows)."""
    return _m_of().T.reshape(-1)


def _contraction_perm_pmajor():
    """perm[p*64+q] = node index m(p, q) (p-major, for x2dh rows)."""
    return _m_of().reshape(-1)


def prepare_inputs(x, a, w1, b1, w2, b2, fw1, fb1, fw2, fb2, fw3, fb3):
    """Shard + re-layout the full model inputs into 8 per-core input maps."""
    x = np.asarray(x, np.float32)
    a = np.asarray(a, np.float32)
    w1 = np.asarray(w1, np.float32)
    w2 = np.asarray(w2, np.float32)
    fw1 = np.asarray(fw1, np.float32)

    # node-major [N, B*F_IN]; x2dh is permuted to "(p q)" contraction order
    x2d = x.transpose(1, 0, 2).reshape(N, C1)
    x2dh = np.ascontiguousarray(
        x2d[_contraction_perm_pmajor()]
    ).astype(np.float16)

    # block-diagonal conv weights (batch-expanded), fp16
    w1bd = np.zeros((K, C1, C2), np.float32)
    for b in range(B):
        w1bd[:, b * F_IN:(b + 1) * F_IN, b * F1:(b + 1) * F1] = w1
    w1bd = np.ascontiguousarray(
        w1bd.transpose(1, 0, 2).reshape(C1, K * C2)
    ).astype(np.float16)
    w2bd = np.zeros((K, P, P), np.float32)
    for c in range(4):
        w2bd[:, c * F1:(c + 1) * F1, c * F2:(c + 1) * F2] = w2
    w2bd = np.ascontiguousarray(
        w2bd.transpose(1, 0, 2).reshape(P, K * P)
    ).astype(np.float16)

    b1r = np.broadcast_to(np.tile(np.asarray(b1, np.float32), B), (P, C2)).copy()
    b2r = np.broadcast_to(np.tile(np.asarray(b2, np.float32), B), (P, C2)).copy()
    fb1r = np.broadcast_to(np.asarray(fb1, np.float32), (B, M1)).copy()
    fb2r = np.broadcast_to(np.asarray(fb2, np.float32), (B, M2)).copy()
    fb3r = np.broadcast_to(np.asarray(fb3, np.float32), (B, M3)).copy()
    fw2_c = np.asarray(fw2, np.float32)
    fw3_c = np.asarray(fw3, np.float32)

    fw1_4 = fw1.reshape(NCORES, NJ, P, F2, M1)
    perm = _contraction_perm()

    in_maps = []
    for i in range(NCORES):
        r0 = i * R
        at_i = np.ascontiguousarray(a[r0:r0 + R, :].T)          # [8192, R], row m
        at4 = np.ascontiguousarray(at_i[perm]).astype(np.float16)
        # local x rows [p, (j f)] with node r0 + j*128 + p
        xloc = np.ascontiguousarray(
            x2d[r0:r0 + R].reshape(NJ, P, C1).transpose(1, 0, 2).reshape(P, NJ * C1)
        ).astype(np.float16)
        # fc1 weight shard, laid out [j, f2_blk, blk, p, m] so each DMA is contiguous
        fw1s = np.ascontiguousarray(
            fw1_4[i].transpose(0, 2, 1, 3).reshape(NJ * F2 * P, M1)
        ).astype(np.float16)
        in_maps.append(
            {
                "at4": at4,
                "x2dh": x2dh,
                "xloc": xloc,
                "w1bd": w1bd,
                "w2bd": w2bd,
                "fw1s": fw1s,
                "fw2": fw2_c,
                "fw3": fw3_c,
                "b1r": b1r,
                "b2r": b2r,
                "fb1r": fb1r,
                "fb2r": fb2r,
                "fb3r": fb3r,
            }
        )
    return in_maps


def kernel(**inputs) -> np.ndarray:
    from concourse.bass_utils import run_bass_kernel_spmd

    if "nc" not in _CACHE:
        _CACHE["nc"] = build_kernel()
    nc = _CACHE["nc"]

    in_maps = prepare_inputs(**inputs)
    res = run_bass_kernel_spmd(nc, in_maps, core_ids=list(range(NCORES)))
    return np.asarray(res.results[0]["out"], np.float32)


if __name__ == "__main__":
    import importlib.util

    spec = importlib.util.spec_from_file_location(
        "reference", os.path.join(os.path.dirname(__file__), "reference.py")
    )
    ref = importlib.util.module_from_spec(spec)
    spec.loader.exec_module(ref)
    inputs = {k: np.asarray(v) for k, v in ref.setup_inputs().items()}
    out = kernel(**inputs)
    print(out)
